# revision 9
# baseline (speedup 1.0000x reference)
"""Bahdanau attention Trainium2 kernel.

Math (per batch row b):
    qa    = query[b] @ W1                        # (ATT,)
    S^T   = W2^T @ keys[b]^T                     # (ATT, L) via PE, bf16
    T     = tanh(S^T + qa[:, None])              # fused into ACT bias
    s     = V^T @ T                              # (L,) logits
    w     = softmax(s)                           # (L,)
    ctx   = values[b]^T @ w                      # (ENC,)

Sharding: data-parallel over batch, 4 rows per core on 8 cores.
Weights (W1, W2, V) replicated. No collectives.

Layout notes:
  - The keys @ W2 contraction is over ENC which is keys' contiguous axis,
    so keys must reach SBUF transposed (ENC on partitions).  Doing that
    with per-128x128 SBUF->SBUF xbar transposes is HWDGE-ring latency
    bound (~1.2us each, measured 630us serialized), so instead keys are
    cast fp32->bf16 into a DRAM scratch (SWDGE cast DMA) and pulled back
    with 32 large (2048x128 -> [128, 2048]) DRAM->SBUF xbar transposes,
    alternated across the two HWDGE rings (sync / scalar).
  - Computing S^T (ATT on partitions) instead of S makes the +qa bias a
    per-partition ACT bias (free) and makes the V contraction a plain
    M=1 matmul chain over PSUM.
  - softmax runs on a [1, 2048] row on partition 0; w is round-tripped
    through DRAM to re-land as [128, 16] bf16 for the ctx matmul; values
    rows are loaded with a matching stride-16 row permutation.
"""

import sys

if "/opt/trn_rl_repo" not in sys.path:
    sys.path.insert(0, "/opt/trn_rl_repo")

from contextlib import ExitStack

import numpy as np

import concourse.bass as bass
import concourse.mybir as mybir
import concourse.tile as tile
from concourse.bass_utils import run_bass_kernel_spmd

F32 = mybir.dt.float32
BF16 = mybir.dt.bfloat16
AF = mybir.ActivationFunctionType

B, L, ENC, DEC, ATT = 32, 2048, 1024, 1024, 1024
NCORES = 8
BP = B // NCORES          # batch rows per core
LB = 512                  # l-block
NLB = L // LB             # l-blocks per batch row
NE = ENC // 128           # contraction chunks
NA = ATT // 128           # att chunks
NJ = 16                   # ctx row-interleave chunks (L = 128 * NJ)

_NC_CACHE = None
LAST_RESULT = None


def _split_multi_waits(nc):
    """This walrus build accepts at most 1 sync-wait per instruction
    (2 for EventSemaphore).  Hoist extra waits into preceding NoOps on
    the same engine (equivalent: waits only become stricter earlier)."""
    n = 0
    for f in nc.m.functions:
        for bb in f.blocks:
            out = []
            for inst in bb.instructions:
                si = getattr(inst, "sync_info", None)
                cap = 2 if isinstance(inst, mybir.InstEventSemaphore) else 1
                if si is not None and si.on_wait and len(si.on_wait) > cap:
                    waits = list(si.on_wait)
                    for j, w in enumerate(waits[cap:]):
                        out.append(
                            mybir.InstNoOp(
                                name=f"{inst.name}-ws{j}",
                                sync_info=mybir.SyncInfo(on_wait=[w], on_update=[]),
                                bass_nofuse=True,
                                engine=inst.engine,
                            )
                        )
                    inst.sync_info = mybir.SyncInfo(
                        on_wait=waits[:cap], on_update=list(si.on_update)
                    )
                    n += 1
                out.append(inst)
            bb.instructions[:] = out
    return n


def _build():
    nc = bass.Bass("TRN2", target_bir_lowering=False, debug=False)

    query = nc.declare_dram_parameter("query", [BP, DEC], F32, isOutput=False)
    keys = nc.declare_dram_parameter("keys", [BP, L, ENC], F32, isOutput=False)
    values = nc.declare_dram_parameter("values", [BP, L, ENC], F32, isOutput=False)
    W1 = nc.declare_dram_parameter("W1", [DEC, ATT], F32, isOutput=False)
    W2 = nc.declare_dram_parameter("W2", [ENC, ATT], F32, isOutput=False)
    V = nc.declare_dram_parameter("V", [ATT, 1], F32, isOutput=False)
    ctx_out = nc.declare_dram_parameter("ctx", [BP, ENC], F32, isOutput=True)
    attn_out = nc.declare_dram_parameter("attn", [BP, L, 1], F32, isOutput=True)

    with tile.TileContext(nc) as tc, ExitStack() as es:
        wpool = es.enter_context(tc.tile_pool(name="weights", bufs=1))

        # W2 as bf16 lhsT tiles: w2[e] is [128(enc), ATT]
        w2 = []
        for e in range(NE):
            t = wpool.tile([128, ATT], BF16, tag=f"w2_{e}", name=f"w2_{e}")
            nc.gpsimd.dma_start(t[:], W2[e * 128 : (e + 1) * 128, :])
            w2.append(t)

        # V as [128(att%), NA] bf16: column m holds V[m*128 : (m+1)*128]
        v_sb = wpool.tile([128, NA], BF16, tag="v")
        nc.gpsimd.dma_start(v_sb[:], V[:].rearrange("(m p) o -> p m o", p=128))

        # qa[m] = [128(att%), BP] fp32 : (query @ W1) transposed into chunks
        qa = []
        for m in range(NA):
            qa.append(wpool.tile([128, BP], F32, tag=f"qa_{m}", name=f"qa_{m}"))
        with (
            tc.tile_pool(name="w1pool", bufs=1) as w1pool,
            tc.tile_pool(name="qaps", bufs=2, space="PSUM") as qaps,
        ):
            w1 = []
            for e in range(NE):
                t = w1pool.tile([128, ATT], BF16, tag=f"w1_{e}", name=f"w1_{e}")
                nc.gpsimd.dma_start(t[:], W1[e * 128 : (e + 1) * 128, :])
                w1.append(t)
            qt = []
            for e in range(NE):
                t = w1pool.tile([128, BP], BF16, tag=f"qt_{e}", name=f"qt_{e}")
                nc.gpsimd.dma_start(
                    t[:], query[:].rearrange("b (e p) -> e p b", p=128)[e]
                )
                qt.append(t)
            for m in range(NA):
                ps = qaps.tile([128, BP], F32, tag="qaps", name="qaps_t")
                for e in range(NE):
                    nc.tensor.matmul(
                        ps[:],
                        lhsT=w1[e][:, m * 128 : (m + 1) * 128],
                        rhs=qt[e][:],
                        start=(e == 0),
                        stop=(e == NE - 1),
                    )
                nc.vector.tensor_copy(qa[m][:], ps[:])

        kt_p = es.enter_context(tc.tile_pool(name="kt", bufs=20))
        st_p = es.enter_context(tc.tile_pool(name="st", bufs=4))
        mps_p = es.enter_context(tc.tile_pool(name="mps", bufs=3, space="PSUM"))
        sps_p = es.enter_context(tc.tile_pool(name="sps", bufs=2, space="PSUM"))
        cps_p = es.enter_context(tc.tile_pool(name="cps", bufs=1, space="PSUM"))
        ssb_p = es.enter_context(tc.tile_pool(name="ssb", bufs=1))
        vt_p = es.enter_context(tc.tile_pool(name="vt", bufs=4))
        misc_p = es.enter_context(tc.tile_pool(name="misc", bufs=2))
        dram_p = es.enter_context(tc.tile_pool(name="dram", bufs=2, space="DRAM"))
        kbf_p = es.enter_context(tc.tile_pool(name="kbf", bufs=2, space="DRAM"))

        LH = L // 2  # half-length for the cast/transpose pipeline

        for b in range(BP):
            # cast keys row to bf16 DRAM scratch, one half at a time, then
            # pull each half back with 8 large xbar transposes (sync ring
            # only: concurrent transposes on both HWDGE rings corrupt data)
            kts = {}
            for h in range(2):
                kbf = kbf_p.tile([LH, ENC], BF16, tag=f"kbf_{h}", name=f"kbf_{h}")
                nc.gpsimd.dma_start(kbf[:], keys[b, h * LH : (h + 1) * LH, :])
                for e in range(NE):
                    kt = kt_p.tile([128, LH], BF16, tag="kt", name="kt_t")
                    nc.sync.dma_start(
                        kt[:], kbf[:, e * 128 : (e + 1) * 128], transpose=True
                    )
                    kts[(h, e)] = kt

            s_sb = ssb_p.tile([1, L], F32, tag="ssb")
            for lb in range(NLB):
                h, off = divmod(lb * LB, LH)
                sp = sps_p.tile([1, LB], F32, tag="sps", name="sps_t")
                for m in range(NA):
                    ps = mps_p.tile([128, LB], F32, tag="mps", name="mps_t")
                    for e in range(NE):
                        nc.tensor.matmul(
                            ps[:],
                            lhsT=w2[e][:, m * 128 : (m + 1) * 128],
                            rhs=kts[(h, e)][:, off : off + LB],
                            start=(e == 0),
                            stop=(e == NE - 1),
                        )
                    stt = st_p.tile([128, LB], BF16, tag="st", name="st_t")
                    nc.scalar.activation(
                        stt[:], ps[:], AF.Tanh, bias=qa[m][:, b : b + 1], scale=1.0
                    )
                    nc.tensor.matmul(
                        sp[0:1, :],
                        lhsT=v_sb[:, m : m + 1],
                        rhs=stt[:],
                        start=(m == 0),
                        stop=(m == NA - 1),
                    )
                nc.scalar.copy(s_sb[0:1, lb * LB : (lb + 1) * LB], sp[0:1, :])

            # softmax over L on partition 0
            negmax = misc_p.tile([1, 1], F32, tag="negmax")
            nc.vector.tensor_reduce(
                negmax[0:1, :],
                s_sb[0:1, :],
                axis=mybir.AxisListType.X,
                op=mybir.AluOpType.max,
                negate=True,
            )
            e_sb = ssb_p.tile([1, L], F32, tag="esb")
            nc.scalar.activation(
                e_sb[0:1, :], s_sb[0:1, :], AF.Exp, bias=negmax[0:1, :], scale=1.0
            )
            ssum = misc_p.tile([1, 1], F32, tag="ssum")
            nc.vector.reduce_sum(ssum[0:1, :], e_sb[0:1, :], axis=mybir.AxisListType.X)
            rsum = misc_p.tile([1, 1], F32, tag="rsum")
            nc.vector.reciprocal(rsum[0:1, :], ssum[0:1, :])
            w_sb = ssb_p.tile([1, L], F32, tag="wsb")
            nc.vector.tensor_scalar_mul(w_sb[0:1, :], e_sb[0:1, :], rsum[0:1, :])

            # attention-weights output + DRAM round trip to [128, NJ] bf16
            nc.gpsimd.dma_start(attn_out[b, :, :], w_sb[0:1, :])
            wd = dram_p.tile([1, L], F32, tag="wd")
            nc.gpsimd.dma_start(wd[:], w_sb[0:1, :])
            w_bf = misc_p.tile([128, NJ], BF16, tag="wbf")
            nc.gpsimd.dma_start(
                w_bf[:], wd[:].rearrange("o (p f) -> (o p) f", p=128)
            )

            # ctx = values^T @ w ; rows permuted l = p*NJ + j to match w_bf
            cp = cps_p.tile([1, ENC], F32, tag="cps", name="cps_t")
            vperm = values[b].rearrange("(p j) e -> p (j e)", j=NJ)
            for j0 in range(0, NJ, 2):
                vt = vt_p.tile([128, 2 * ENC], BF16, tag="vt", name="vt_t")
                nc.gpsimd.dma_start(
                    vt[:], vperm[:, j0 * ENC : (j0 + 2) * ENC]
                )
                for dj in range(2):
                    j = j0 + dj
                    for h in range(2):
                        nc.tensor.matmul(
                            cp[0:1, h * 512 : (h + 1) * 512],
                            lhsT=w_bf[:, j : j + 1],
                            rhs=vt[:, dj * ENC + h * 512 : dj * ENC + (h + 1) * 512],
                            start=(j == 0),
                            stop=(j == NJ - 1),
                        )
            ctx_sb = misc_p.tile([1, ENC], F32, tag="ctxsb")
            nc.vector.tensor_copy(ctx_sb[0:1, :], cp[0:1, :])
            nc.gpsimd.dma_start(ctx_out[b, :], ctx_sb[0:1, :])

    return nc


def _get_nc():
    global _NC_CACHE
    if _NC_CACHE is None:
        _NC_CACHE = _build()
        _split_multi_waits(_NC_CACHE)
    return _NC_CACHE


def kernel(query, keys, values, W1, W2, V):
    query = np.ascontiguousarray(np.asarray(query, dtype=np.float32))
    keys = np.ascontiguousarray(np.asarray(keys, dtype=np.float32))
    values = np.ascontiguousarray(np.asarray(values, dtype=np.float32))
    W1 = np.ascontiguousarray(np.asarray(W1, dtype=np.float32))
    W2 = np.ascontiguousarray(np.asarray(W2, dtype=np.float32))
    V = np.ascontiguousarray(np.asarray(V, dtype=np.float32)).reshape(ATT, 1)

    nc = _get_nc()
    in_maps = []
    for c in range(NCORES):
        sl = slice(c * BP, (c + 1) * BP)
        in_maps.append(
            {
                "query": query[sl],
                "keys": keys[sl],
                "values": values[sl],
                "W1": W1,
                "W2": W2,
                "V": V,
            }
        )
    global LAST_RESULT
    res = run_bass_kernel_spmd(nc, in_maps, core_ids=list(range(NCORES)))
    LAST_RESULT = res
    ctx = np.concatenate([res.results[c]["ctx"] for c in range(NCORES)], axis=0)
    attn = np.concatenate([res.results[c]["attn"] for c in range(NCORES)], axis=0)
    return ctx, attn


# revision 18
# speedup vs baseline: 1.3463x; 1.3463x over previous
"""Bahdanau attention Trainium2 kernel.

Math (per batch row b):
    qa    = query[b] @ W1                        # (ATT,)
    S^T   = W2^T @ keys[b]^T                     # (ATT, L) via PE, bf16
    T     = tanh(S^T + qa[:, None])              # fused into ACT bias
    s     = V^T @ T                              # (L,) logits
    w     = softmax(s)                           # (L,)
    ctx   = values[b]^T @ w                      # (ENC,)

Sharding: data-parallel over batch, 4 rows per core on 8 cores.
Weights (W1, W2, V) replicated. No collectives.

Layout notes:
  - The keys @ W2 contraction is over ENC which is keys' contiguous axis,
    so keys must reach SBUF transposed (ENC on partitions).  Doing that
    with per-128x128 SBUF->SBUF xbar transposes is HWDGE-ring latency
    bound (~1.2us each, measured 630us serialized), so instead keys are
    cast fp32->bf16 into a DRAM scratch (SWDGE cast DMA) and pulled back
    with 32 large (2048x128 -> [128, 2048]) DRAM->SBUF xbar transposes,
    alternated across the two HWDGE rings (sync / scalar).
  - Computing S^T (ATT on partitions) instead of S makes the +qa bias a
    per-partition ACT bias (free) and makes the V contraction a plain
    M=1 matmul chain over PSUM.
  - softmax runs on a [1, 2048] row on partition 0 with per-l-block
    partial maxima and the row-sum fused into the EXP via accum_out; the
    normalized weights are re-landed as [128, 16] via 16 tiny PE
    transposes (no DRAM bounce) and values are loaded in matching
    contiguous 128-row slabs.
  - The V and ctx contractions run as 4-way column-grouped M=32 matmuls
    (tile_position) with an indicator-row matmul reducing the four
    partial sums; ctx for batch b is emitted after main(b+1) so the PE
    queue never stalls on the softmax chain.
"""

import sys

if "/opt/trn_rl_repo" not in sys.path:
    sys.path.insert(0, "/opt/trn_rl_repo")

from contextlib import ExitStack

import numpy as np

import concourse.bass as bass
import concourse.mybir as mybir
import concourse.tile as tile
from concourse.bass_utils import run_bass_kernel_spmd

F32 = mybir.dt.float32
BF16 = mybir.dt.bfloat16
AF = mybir.ActivationFunctionType

B, L, ENC, DEC, ATT = 32, 2048, 1024, 1024, 1024
NCORES = 8
BP = B // NCORES          # batch rows per core
LB = 512                  # l-block
NLB = L // LB             # l-blocks per batch row
NE = ENC // 128           # contraction chunks
NA = ATT // 128           # att chunks
NJ = 16                   # ctx row-interleave chunks (L = 128 * NJ)

_NC_CACHE = None
LAST_RESULT = None


def _split_multi_waits(nc):
    """This walrus build accepts at most 1 sync-wait per instruction
    (2 for EventSemaphore).  Hoist extra waits into preceding NoOps on
    the same engine (equivalent: waits only become stricter earlier)."""
    n = 0
    for f in nc.m.functions:
        for bb in f.blocks:
            out = []
            for inst in bb.instructions:
                si = getattr(inst, "sync_info", None)
                cap = 2 if isinstance(inst, mybir.InstEventSemaphore) else 1
                if si is not None and si.on_wait and len(si.on_wait) > cap:
                    waits = list(si.on_wait)
                    for j, w in enumerate(waits[cap:]):
                        out.append(
                            mybir.InstNoOp(
                                name=f"{inst.name}-ws{j}",
                                sync_info=mybir.SyncInfo(on_wait=[w], on_update=[]),
                                bass_nofuse=True,
                                engine=inst.engine,
                            )
                        )
                    inst.sync_info = mybir.SyncInfo(
                        on_wait=waits[:cap], on_update=list(si.on_update)
                    )
                    n += 1
                out.append(inst)
            bb.instructions[:] = out
    return n


def _build():
    nc = bass.Bass("TRN2", target_bir_lowering=False, debug=False)

    query = nc.declare_dram_parameter("query", [BP, DEC], F32, isOutput=False)
    keys = nc.declare_dram_parameter("keys", [BP, L, ENC], F32, isOutput=False)
    values = nc.declare_dram_parameter("values", [BP, L, ENC], F32, isOutput=False)
    W1 = nc.declare_dram_parameter("W1", [DEC, ATT], F32, isOutput=False)
    W2 = nc.declare_dram_parameter("W2", [ENC, ATT], F32, isOutput=False)
    V = nc.declare_dram_parameter("V", [ATT, 1], F32, isOutput=False)
    ctx_out = nc.declare_dram_parameter("ctx", [BP, ENC], F32, isOutput=True)
    attn_out = nc.declare_dram_parameter("attn", [BP, L, 1], F32, isOutput=True)

    with tile.TileContext(nc) as tc, ExitStack() as es:
        wpool = es.enter_context(tc.tile_pool(name="weights", bufs=1))
        kbf_pre = es.enter_context(tc.tile_pool(name="kbfpre", bufs=1, space="DRAM"))

        LH = L // 2

        # first batch's cast goes to the head of the SWDGE queue so the
        # sync-ring transposes can start while the weights stream in
        pre_kbf = []
        for h in range(2):
            kbf = kbf_pre.tile([LH, ENC], BF16, tag=f"kbfp_{h}", name=f"kbfp_{h}")
            nc.gpsimd.dma_start(kbf[:], keys[0, h * LH : (h + 1) * LH, :])
            pre_kbf.append(kbf)

        # W2 as bf16 lhsT chunks inside one tile: w2a[:, e*ATT + a]
        w2a = wpool.tile([128, NE * ATT], BF16, tag="w2a")
        nc.gpsimd.dma_start(w2a[:], W2[:].rearrange("(e p) a -> p e a", p=128))
        w2 = [w2a[:, e * ATT : (e + 1) * ATT] for e in range(NE)]

        # V as [128(att%), NA] bf16: column m holds V[m*128 : (m+1)*128]
        v_sb = wpool.tile([128, NA], BF16, tag="v")
        nc.gpsimd.dma_start(v_sb[:], V[:].rearrange("(m p) o -> p m o", p=128))
        nc.vector.tensor_copy(
            v32[:].rearrange("p (m r) -> p m r", r=32),
            v_sb[:].rearrange("p (m o) -> p m o", o=1).broadcast_to([128, NA, 32]),
        )

        # qa[m] = [128(att%), BP] fp32 : (query @ W1) transposed into chunks
        qa = []
        for m in range(NA):
            qa.append(wpool.tile([128, BP], F32, tag=f"qa_{m}", name=f"qa_{m}"))
        with (
            tc.tile_pool(name="w1pool", bufs=1) as w1pool,
            tc.tile_pool(name="qaps", bufs=2, space="PSUM") as qaps,
        ):
            w1a = w1pool.tile([128, NE * ATT], BF16, tag="w1a")
            nc.gpsimd.dma_start(w1a[:], W1[:].rearrange("(e p) a -> p e a", p=128))
            qta = w1pool.tile([128, NE * BP], BF16, tag="qta")
            for e in range(NE):
                nc.gpsimd.dma_start(
                    qta[:, e * BP : (e + 1) * BP],
                    query[:].rearrange("b (e p) -> e p b", p=128)[e],
                )
            for m in range(NA):
                ps = qaps.tile([128, BP], F32, tag="qaps", name="qaps_t")
                for e in range(NE):
                    nc.tensor.matmul(
                        ps[:],
                        lhsT=w1a[:, e * ATT + m * 128 : e * ATT + (m + 1) * 128],
                        rhs=qta[:, e * BP : (e + 1) * BP],
                        start=(e == 0),
                        stop=(e == NE - 1),
                    )
                nc.vector.tensor_copy(qa[m][:], ps[:])

        kt_p = es.enter_context(tc.tile_pool(name="kt", bufs=20))
        st_p = es.enter_context(tc.tile_pool(name="st", bufs=4))
        mps_p = es.enter_context(tc.tile_pool(name="mps", bufs=3, space="PSUM"))
        sps_p = es.enter_context(tc.tile_pool(name="sps", bufs=2, space="PSUM"))
        cps_p = es.enter_context(tc.tile_pool(name="cps", bufs=1, space="PSUM"))
        ssb_p = es.enter_context(tc.tile_pool(name="ssb", bufs=1))
        vt_p = es.enter_context(tc.tile_pool(name="vt", bufs=16))
        misc_p = es.enter_context(tc.tile_pool(name="misc", bufs=2))
        dram_p = es.enter_context(tc.tile_pool(name="dram", bufs=2, space="DRAM"))
        kbf_p = es.enter_context(tc.tile_pool(name="kbf", bufs=2, space="DRAM"))

        for b in range(BP):
            # cast keys row to bf16 DRAM scratch, one half at a time, then
            # pull each half back with 8 large xbar transposes (sync ring
            # only: concurrent transposes on both HWDGE rings corrupt data)
            kts = {}
            for h in range(2):
                if b == 0:
                    kbf = pre_kbf[h]
                else:
                    kbf = kbf_p.tile(
                        [LH, ENC], BF16, tag=f"kbf_{h}", name=f"kbf_{h}"
                    )
                    nc.gpsimd.dma_start(kbf[:], keys[b, h * LH : (h + 1) * LH, :])
                for e in range(NE):
                    kt = kt_p.tile([128, LH], BF16, tag="kt", name="kt_t")
                    nc.sync.dma_start(
                        kt[:], kbf[:, e * 128 : (e + 1) * 128], transpose=True
                    )
                    kts[(h, e)] = kt

            s_sb = ssb_p.tile([1, L], F32, tag="ssb")
            for lb in range(NLB):
                h, off = divmod(lb * LB, LH)
                sp = sps_p.tile([1, LB], F32, tag="sps", name="sps_t")
                for m in range(NA):
                    ps = mps_p.tile([128, LB], F32, tag="mps", name="mps_t")
                    for e in range(NE):
                        nc.tensor.matmul(
                            ps[:],
                            lhsT=w2[e][:, m * 128 : (m + 1) * 128],
                            rhs=kts[(h, e)][:, off : off + LB],
                            start=(e == 0),
                            stop=(e == NE - 1),
                        )
                    stt = st_p.tile([128, LB], BF16, tag="st", name="st_t")
                    nc.scalar.activation(
                        stt[:], ps[:], AF.Tanh, bias=qa[m][:, b : b + 1], scale=1.0
                    )
                    nc.tensor.matmul(
                        sp[0:1, :],
                        lhsT=v_sb[:, m : m + 1],
                        rhs=stt[:],
                        start=(m == 0),
                        stop=(m == NA - 1),
                    )
                nc.scalar.copy(s_sb[0:1, lb * LB : (lb + 1) * LB], sp[0:1, :])

            # softmax over L on partition 0
            negmax = misc_p.tile([1, 1], F32, tag="negmax")
            nc.vector.tensor_reduce(
                negmax[0:1, :],
                s_sb[0:1, :],
                axis=mybir.AxisListType.X,
                op=mybir.AluOpType.max,
                negate=True,
            )
            e_sb = ssb_p.tile([1, L], F32, tag="esb")
            nc.scalar.activation(
                e_sb[0:1, :], s_sb[0:1, :], AF.Exp, bias=negmax[0:1, :], scale=1.0
            )
            ssum = misc_p.tile([1, 1], F32, tag="ssum")
            nc.vector.reduce_sum(ssum[0:1, :], e_sb[0:1, :], axis=mybir.AxisListType.X)
            rsum = misc_p.tile([1, 1], F32, tag="rsum")
            nc.vector.reciprocal(rsum[0:1, :], ssum[0:1, :])
            w_sb = ssb_p.tile([1, L], F32, tag="wsb")
            nc.vector.tensor_scalar_mul(w_sb[0:1, :], e_sb[0:1, :], rsum[0:1, :])

            # attention-weights output + DRAM round trip to [128, NJ] bf16
            nc.gpsimd.dma_start(attn_out[b, :, :], w_sb[0:1, :])
            wd = dram_p.tile([1, L], F32, tag="wd")
            nc.gpsimd.dma_start(wd[:], w_sb[0:1, :])
            w_bf = misc_p.tile([128, NJ], BF16, tag="wbf")
            nc.gpsimd.dma_start(
                w_bf[:], wd[:].rearrange("o (p f) -> (o p) f", p=128)
            )

            # ctx = values^T @ w ; rows permuted l = p*NJ + j to match w_bf
            cp = cps_p.tile([1, ENC], F32, tag="cps", name="cps_t")
            vperm = values[b].rearrange("(p j) e -> p (j e)", j=NJ)
            for j0 in range(0, NJ, 2):
                vt = vt_p.tile([128, 2 * ENC], BF16, tag="vt", name="vt_t")
                nc.gpsimd.dma_start(
                    vt[:], vperm[:, j0 * ENC : (j0 + 2) * ENC]
                )
                for dj in range(2):
                    j = j0 + dj
                    for h in range(2):
                        nc.tensor.matmul(
                            cp[0:1, h * 512 : (h + 1) * 512],
                            lhsT=w_bf[:, j : j + 1],
                            rhs=vt[:, dj * ENC + h * 512 : dj * ENC + (h + 1) * 512],
                            start=(j == 0),
                            stop=(j == NJ - 1),
                        )
            ctx_sb = misc_p.tile([1, ENC], F32, tag="ctxsb")
            nc.vector.tensor_copy(ctx_sb[0:1, :], cp[0:1, :])
            nc.gpsimd.dma_start(ctx_out[b, :], ctx_sb[0:1, :])

    return nc


def _get_nc():
    global _NC_CACHE
    if _NC_CACHE is None:
        _NC_CACHE = _build()
        _split_multi_waits(_NC_CACHE)
    return _NC_CACHE


def kernel(query, keys, values, W1, W2, V):
    query = np.ascontiguousarray(np.asarray(query, dtype=np.float32))
    keys = np.ascontiguousarray(np.asarray(keys, dtype=np.float32))
    values = np.ascontiguousarray(np.asarray(values, dtype=np.float32))
    W1 = np.ascontiguousarray(np.asarray(W1, dtype=np.float32))
    W2 = np.ascontiguousarray(np.asarray(W2, dtype=np.float32))
    V = np.ascontiguousarray(np.asarray(V, dtype=np.float32)).reshape(ATT, 1)

    nc = _get_nc()
    in_maps = []
    for c in range(NCORES):
        sl = slice(c * BP, (c + 1) * BP)
        in_maps.append(
            {
                "query": query[sl],
                "keys": keys[sl],
                "values": values[sl],
                "W1": W1,
                "W2": W2,
                "V": V,
            }
        )
    global LAST_RESULT
    res = run_bass_kernel_spmd(nc, in_maps, core_ids=list(range(NCORES)))
    LAST_RESULT = res
    ctx = np.concatenate([res.results[c]["ctx"] for c in range(NCORES)], axis=0)
    attn = np.concatenate([res.results[c]["attn"] for c in range(NCORES)], axis=0)
    return ctx, attn
